# revision 23
# baseline (speedup 1.0000x reference)
"""Trainium2 Bass kernel for nn_NoBrainEncoderBlock_31662498906140.

out[b] = softmax_n( clip( cos(q1[b], k1[b,n]) * mask[b,n], 0, 1 ) )

Only q1, k1, mask affect the output (q2, k2, temp are unused by the math).
q1 is L2-normalized on the host (256 KiB — negligible), so the device
program computes, per batch row b and key row n:

    dot[n]  = sum_d k1[b,n,d] * q1n[b,d]        (VectorE tensor_tensor_reduce)
    ss[n]   = sum_d k1[b,n,d]^2                 (ScalarE activation Square+accum)
    score   = clip(dot * rsqrt(ss) * mask, 0, 1)
    out     = exp(score) / sum_n exp(score)     (scores in [0,1] -> exp safe)

Sharding: data-parallel over batch B=32 across 8 NeuronCores (4 rows per
core, 64 MiB of k1 per core). k1 is streamed HBM->SBUF once in 4 MiB
chunks (HWDGE); the two fused reduction passes run on different engines
(DVE + ACT) so both overlap the DMA stream. No cross-core communication.
"""

import numpy as np

B, N, D = 32, 2048, 2048
NCORES = 8
BPC = B // NCORES          # 4 batch rows per core
T = N // 128               # 16 column-chunks of 128 key rows
KC = 4                     # key chunks per DMA (4 MiB transfers)

_CACHE = {}


def _build_nc(reps=1, kc=KC, kbufs=3, scr_bf16=False, dma_alt=False, k_bf16=False):
    """Build the per-core program. reps>1 unrolls the whole computation
    multiple times inside one NEFF — used only for device-time measurement
    (slope over reps cancels per-launch overhead)."""
    import concourse.bacc as bacc
    import concourse.tile as tile
    from concourse import mybir

    f32 = mybir.dt.float32
    Alu = mybir.AluOpType
    Act = mybir.ActivationFunctionType

    nc = bacc.Bacc()
    qn = nc.declare_dram_parameter("qn", [BPC, D], f32, isOutput=False)
    k1 = nc.declare_dram_parameter("k1", [BPC, N, D], f32, isOutput=False)
    mask = nc.declare_dram_parameter("mask", [BPC, N], f32, isOutput=False)
    out = nc.declare_dram_parameter("out", [BPC, N], f32, isOutput=True)

    with tile.TileContext(nc) as tc:
        with (
            tc.tile_pool(name="kpool", bufs=kbufs) as kpool,
            tc.tile_pool(name="qpool", bufs=1) as qpool,
            tc.tile_pool(name="spool", bufs=2) as spool,
            tc.tile_pool(name="stats", bufs=2) as stats,
        ):
            # Broadcast each (pre-normalized) q row across all 128 partitions.
            kdt = mybir.dt.bfloat16 if k_bf16 else f32
            qbs = []
            for b in range(BPC):
                qb = qpool.tile([128, D], kdt, name=f"qb{b}", tag=f"qb{b}")
                nc.gpsimd.dma_start(
                    out=qb[:], in_=qn[b : b + 1, :].to_broadcast([128, D])
                )
                qbs.append(qb)

            for _rep in range(reps):
                _kernel_body(
                    nc, tc, kpool, spool, stats, qbs, qn, k1, mask, out,
                    kc, scr_bf16, dma_alt, k_bf16,
                )

    nc.finalize()
    return nc


def _kernel_body(
    nc, tc, kpool, spool, stats, qbs, qn, k1, mask, out, kc, scr_bf16,
    dma_alt=False, k_bf16=False,
):
    from concourse import mybir
    import concourse.bass_isa as bass_isa

    f32 = mybir.dt.float32
    sdt = mybir.dt.bfloat16 if (scr_bf16 or k_bf16) else f32
    kdt = mybir.dt.bfloat16 if k_bf16 else f32
    Alu = mybir.AluOpType
    Act = mybir.ActivationFunctionType

    if True:
        if True:
            # Per-(partition, batch, chunk) reduction results.
            dots = stats.tile([128, BPC, T], f32, name="dots", tag="dots")
            ssum = stats.tile([128, BPC, T], f32, name="ssum", tag="ssum")

            for b in range(BPC):
                k1b = k1[b, :, :].rearrange("(t p) d -> p t d", p=128)  # [128,T,D]
                for s in range(T // kc):
                    kt = kpool.tile([128, kc, D], kdt, name="kt", tag="kt")
                    if k_bf16:
                        # cast-on-DMA needs SWDGE
                        nc.gpsimd.dma_start(
                            out=kt[:], in_=k1b[:, s * kc : (s + 1) * kc, :]
                        )
                    else:
                        eng = (
                            nc.scalar
                            if (dma_alt and (b * (T // kc) + s) % 2)
                            else nc.sync
                        )
                        eng.dma_start(
                            out=kt[:], in_=k1b[:, s * kc : (s + 1) * kc, :]
                        )
                    for c in range(kc):
                        t = s * kc + c
                        dscr = spool.tile([128, D], sdt, name="dscr", tag="dscr")
                        nc.vector.scalar_tensor_tensor(
                            out=dscr[:],
                            in0=kt[:, c, :],
                            scalar=0.0,
                            in1=qbs[b][:],
                            op0=Alu.bypass,
                            op1=Alu.mult,
                            accum_out=dots[:, b, t : t + 1],
                        )
                        ascr = spool.tile([128, D], sdt, name="ascr", tag="ascr")
                        nc.scalar.activation(
                            out=ascr[:],
                            in_=kt[:, c, :],
                            func=Act.Square,
                            accum_out=ssum[:, b, t : t + 1],
                        )

            # Epilogue over the [128, BPC, T] stats (2048 scores per batch).
            kn = stats.tile([128, BPC, T], f32, name="kn", tag="kn")
            nc.scalar.sqrt(kn[:], ssum[:])
            rkn = stats.tile([128, BPC, T], f32, name="rkn", tag="rkn")
            nc.vector.reciprocal(rkn[:], kn[:])

            sc = stats.tile([128, BPC, T], f32, name="sc", tag="sc")
            nc.vector.tensor_mul(sc[:], dots[:], rkn[:])

            mt = stats.tile([128, BPC, T], f32, name="mt", tag="mt")
            nc.gpsimd.dma_start(
                out=mt[:], in_=mask[:, :].rearrange("b (t p) -> p b t", p=128)
            )
            nc.vector.tensor_mul(sc[:], sc[:], mt[:])
            nc.vector.tensor_scalar_max(sc[:], sc[:], 0.0)
            nc.vector.tensor_scalar_min(sc[:], sc[:], 1.0)

            e = stats.tile([128, BPC, T], f32, name="e", tag="e")
            nc.scalar.activation(e[:], sc[:], Act.Exp)

            esum = stats.tile([128, BPC], f32, name="esum", tag="esum")
            nc.vector.tensor_reduce(
                out=esum[:], in_=e[:], axis=mybir.AxisListType.X, op=Alu.add
            )
            stot = stats.tile([128, BPC], f32, name="stot", tag="stot")
            import concourse.bass_isa as bass_isa

            nc.gpsimd.partition_all_reduce(
                stot[:], esum[:], channels=128, reduce_op=bass_isa.ReduceOp.add
            )
            rtot = stats.tile([128, BPC], f32, name="rtot", tag="rtot")
            nc.vector.reciprocal(rtot[:], stot[:])
            for b in range(BPC):
                nc.vector.tensor_scalar_mul(
                    e[:, b, :], e[:, b, :], rtot[:, b : b + 1]
                )

            nc.gpsimd.dma_start(
                out=out[:, :].rearrange("b (t p) -> p b t", p=128), in_=e[:]
            )


def _get_fn(reps=1, **opts):
    key = ("fn", reps, tuple(sorted(opts.items())))
    if key in _CACHE:
        return _CACHE[key]
    import jax
    import jax.numpy as jnp
    from jax.experimental.shard_map import shard_map
    from jax.sharding import Mesh, PartitionSpec

    from concourse import bass2jax, mybir

    bass2jax.install_neuronx_cc_hook()
    nc = _build_nc(reps, **opts)

    in_names = []
    out_names = []
    out_avals = []
    partition_name = nc.partition_id_tensor.name if nc.partition_id_tensor else None
    for alloc in nc.m.functions[0].allocations:
        if not isinstance(alloc, mybir.MemoryLocationSet):
            continue
        name = alloc.memorylocations[0].name
        if alloc.kind == "ExternalInput":
            if name != partition_name:
                in_names.append(name)
        elif alloc.kind == "ExternalOutput":
            shape = tuple(alloc.tensor_shape)
            dtype = mybir.dt.np(alloc.dtype)
            out_names.append(name)
            out_avals.append(jax.core.ShapedArray(shape, dtype))
    n_params = len(in_names)
    all_in_names = list(in_names) + list(out_names)
    if partition_name is not None:
        all_in_names.append(partition_name)

    def _body(*args):
        ops = list(args)
        if partition_name is not None:
            ops.append(bass2jax.partition_id_tensor())
        outs = bass2jax._bass_exec_p.bind(
            *ops,
            out_avals=tuple(out_avals),
            in_names=tuple(all_in_names),
            out_names=tuple(out_names),
            lowering_input_output_aliases=(),
            sim_require_finite=True,
            sim_require_nnan=True,
            nc=nc,
        )
        return tuple(outs)

    devices = jax.devices()[:NCORES]
    mesh = Mesh(np.asarray(devices), ("core",))
    P = PartitionSpec
    fn = jax.jit(
        shard_map(
            _body,
            mesh=mesh,
            in_specs=(P("core"),) * (n_params + len(out_names)),
            out_specs=(P("core"),) * len(out_names),
            check_rep=False,
        )
    )
    _CACHE[key] = (fn, mesh)
    return _CACHE[key]


def _prep_inputs(q1, k1, mask):
    q1 = np.ascontiguousarray(np.asarray(q1, dtype=np.float32))
    k1 = np.ascontiguousarray(np.asarray(k1, dtype=np.float32))
    mask = np.ascontiguousarray(np.asarray(mask, dtype=np.float32))
    qn = q1 / np.maximum(
        np.linalg.norm(q1, axis=-1, keepdims=True), 1e-12
    ).astype(np.float32)
    return qn.astype(np.float32), k1, mask


def _zero_out():
    return np.zeros((B, N), dtype=np.float32)


def kernel(q1, k1, q2, k2, mask, temp=None):
    qn, k1, mask = _prep_inputs(q1, k1, mask)
    fn, _ = _get_fn()
    (out,) = fn(qn, k1, mask, _zero_out())
    return np.asarray(out, dtype=np.float32).reshape(B, N)
